# revision 1
# baseline (speedup 1.0000x reference)
"""3x3 valid conv (single channel) on 8 TRN2 NeuronCores.

Strategy: shard X row-wise (512 rows/core + 2 halo rows). Per core, the conv
is computed as 3 banded matmuls per output tile accumulating in PSUM:
    out[m, c] = sum_dj (B_dj.T @ X_tile[:, c+dj])[m]
where B_dj[k, m] = W[k-m, dj] is a [K, M] banded stationary operand built on
the host from the runtime W. Matmuls run in float32r (TF32-like, 1 cyc/row)
with explicit rounding copies; bias is fused into the PSUM->SBUF drain on the
scalar engine. Memory-bound target: X loaded once (plus 2-row tile halos),
output stored once, DMAs batched >=1 MiB.
"""

import sys

sys.path.insert(0, "/opt/trn_rl_repo")

import numpy as np
from concourse import bass, mybir
from concourse.bass_utils import run_bass_kernel_spmd
from concourse.tile import TileContext

F32 = mybir.dt.float32
F32R = mybir.dt.float32r

H, WIDTH = 4096, 8192
KH, KW = 3, 3
OH, OW = H - KH + 1, WIDTH - KW + 1
N_CORES = 8
RPC = H // N_CORES          # 512 output rows produced per core
IN_ROWS = RPC + KH - 1      # 514 input rows per core (2-row halo)
# (in_row0, store_off, y_row0, store_rows): strips are always 128-row loads
# (DMA balancing needs 128 partitions) producing 126 output rows. The last
# strip covers rows 386..513 and stores only its final 8 rows (504..511); its
# first 120 rows are copied from strip 3's rounded tile in SBUF (saves a
# 3.8 MB HBM re-read), only rows 506..513 come from DRAM.
ROW_TILES = [
    (0, 0, 0, 126),
    (126, 0, 126, 126),
    (252, 0, 252, 126),
    (378, 0, 378, 126),
    (386, 118, 504, 8),
]
N_COL_TILES = 16            # 15 x 512 + 1 x 510 = 8190
COL_GROUPS = 4              # 4 col tiles staged per output DMA (~1 MiB)


def _split_multi_waits(nc, max_waits=1):
    # This container's walrus rejects >1 sync-wait command per instruction
    # (CoreV3 setupSyncWait). Tile attaches one wait per producing logical
    # processor to a single instruction; hoist the excess onto same-engine
    # Drain carriers inserted immediately before it.
    for fn in nc.m.functions:
        for bb in fn.blocks:
            out = []
            changed = False
            for inst in bb.instructions:
                si = inst.sync_info
                waits = list(si.on_wait) if si and si.on_wait else []
                if len(waits) > max_waits:
                    rest = waits[max_waits:]
                    for j in range(0, len(rest), max_waits):
                        carrier = mybir.InstDrain(
                            name=nc.get_next_instruction_name(), ins=[], outs=[]
                        )
                        carrier.engine = inst.engine
                        carrier.sync_info = mybir.SyncInfo(
                            on_wait=rest[j : j + max_waits], on_update=[]
                        )
                        out.append(carrier)
                    si.on_wait = waits[:max_waits]
                    changed = True
                out.append(inst)
            if changed:
                bb.instructions = out


def _build(split_waits=True):
    nc = bass.Bass()
    x = nc.declare_dram_parameter("x", [IN_ROWS, WIDTH], F32, isOutput=False)
    bands = nc.declare_dram_parameter("bands", [128, 3 * 128], F32, isOutput=False)
    bands2 = nc.declare_dram_parameter("bands2", [128, 3 * 128], F32, isOutput=False)
    bias = nc.declare_dram_parameter("bias", [128, 1], F32, isOutput=False)
    y = nc.declare_dram_parameter("y", [RPC, OW], F32, isOutput=True)

    ident = mybir.ActivationFunctionType.Identity

    with TileContext(nc) as tc:
        with (
            tc.tile_pool(name="const", bufs=1) as cpool,
            tc.tile_pool(name="xin", bufs=2) as xpool,
            tc.tile_pool(name="xr", bufs=3) as rpool,
            tc.tile_pool(name="stage", bufs=3) as spool,
            tc.tile_pool(name="psum", bufs=6, space="PSUM") as ppool,
        ):
            band_f = cpool.tile([128, 3 * 128], F32)
            nc.gpsimd.dma_start(out=band_f[:], in_=bands[:])
            band_r = cpool.tile([128, 3 * 128], F32R)
            nc.vector.tensor_copy(band_r[:], band_f[:])
            band2_f = cpool.tile([128, 3 * 128], F32)
            nc.gpsimd.dma_start(out=band2_f[:], in_=bands2[:])
            band2_r = cpool.tile([128, 3 * 128], F32R)
            nc.vector.tensor_copy(band2_r[:], band2_f[:])
            bias_t = cpool.tile([128, 1], F32)
            nc.gpsimd.dma_start(out=bias_t[:], in_=bias[:])

            prev_xt = None
            for strip, (r0, s0, y0, srows) in enumerate(ROW_TILES):
                xr = rpool.tile([128, WIDTH], F32R, tag="xr")
                xt = xpool.tile([128, WIDTH], F32, tag="xt")
                if strip < len(ROW_TILES) - 1:
                    # split the 4 MB load into column halves so the first
                    # matmuls start after ~2 MB instead of 4 MB; col tiles
                    # 0..6 depend only on the first half via subtile deps
                    nc.sync.dma_start(out=xt[:, 0:4096], in_=x[r0 : r0 + 128, 0:4096])
                    nc.vector.tensor_copy(xr[:, 0:4096], xt[:, 0:4096])
                    nc.sync.dma_start(out=xt[:, 4096:WIDTH], in_=x[r0 : r0 + 128, 4096:WIDTH])
                    nc.vector.tensor_copy(xr[:, 4096:WIDTH], xt[:, 4096:WIDTH])
                    band = band_r
                else:
                    # Permuted layout (band2 compensates): partitions 0..7 =
                    # fresh DRAM rows 506..513; partitions 8..127 = rows
                    # 386..505 from strip 3's f32 tile (saves a 3.8 MB HBM
                    # re-read). The SBUF->SBUF copy must span all 128
                    # partitions for balanced DMA (non-128 degenerates badly),
                    # so copy the whole tile in col halves on the idle SWDGE
                    # ring, then overwrite partitions 0..7 with the fresh rows
                    # (issued after => WAW dep orders it). One base-0
                    # full-tile round keeps the DVE partition rule happy.
                    nc.gpsimd.dma_start(out=xt[:, 0:4096], in_=prev_xt[:, 0:4096])
                    nc.gpsimd.dma_start(out=xt[:, 4096:WIDTH], in_=prev_xt[:, 4096:WIDTH])
                    nc.sync.dma_start(out=xt[0:8, :], in_=x[506:514, :])
                    nc.vector.tensor_copy(xr[:, :], xt[:, :])
                    band = band2_r
                prev_xt = xt

                for g in range(COL_GROUPS):
                    gw = 2048 if g < COL_GROUPS - 1 else 2046
                    stage = spool.tile([128, 2048], F32, tag="stage")
                    for j in range(N_COL_TILES // COL_GROUPS):
                        ct = g * 4 + j
                        c0 = ct * 512
                        n = 512 if ct < N_COL_TILES - 1 else 510
                        ps = ppool.tile([128, 512], F32, tag="ps")
                        for dj in range(KW):
                            nc.tensor.matmul(
                                ps[:126, :n],
                                band[:, dj * 128 : dj * 128 + 126],
                                xr[:, c0 + dj : c0 + dj + n],
                                start=(dj == 0),
                                stop=(dj == KW - 1),
                            )
                        nc.scalar.activation(
                            stage[:126, j * 512 : j * 512 + n],
                            ps[:126, :n],
                            ident,
                            bias=bias_t[:126, :],
                            scale=1.0,
                        )
                    # stores ride the ACT HWDGE ring so the multi-MB loads on
                    # the SP ring can't head-of-line-block them
                    nc.scalar.dma_start(
                        out=y[y0 : y0 + srows, g * 2048 : g * 2048 + gw],
                        in_=stage[s0 : s0 + srows, :gw],
                    )

    if split_waits:
        _split_multi_waits(nc)
    return nc


_NC_CACHE = None


def _get_nc():
    global _NC_CACHE
    if _NC_CACHE is None:
        _NC_CACHE = _build()
    return _NC_CACHE


def _make_host_inputs(X, W, b):
    X = np.ascontiguousarray(np.asarray(X, dtype=np.float32))
    W = np.asarray(W, dtype=np.float32)
    b = np.asarray(b, dtype=np.float32)

    bands = np.zeros((128, 3 * 128), dtype=np.float32)
    for dj in range(KW):
        for dk in range(KH):
            # B_dj[m+dk, m] = W[dk, dj] for every output row m
            mm = np.arange(126)
            bands[mm + dk, dj * 128 + mm] = W[dk, dj]
    # strip-4 permuted band: partition k holds input local row 506+k (k<8)
    # or 378+k (k>=8); band col m is output local row 386+m
    bands2 = np.zeros((128, 3 * 128), dtype=np.float32)
    for dj in range(KW):
        for k in range(128):
            row = 506 + k if k < 8 else 378 + k
            for dk in range(KH):
                m = row - dk - 386
                if 0 <= m < 126:
                    bands2[k, dj * 128 + m] = W[dk, dj]
    bias = np.full((128, 1), float(b[0]), dtype=np.float32)

    in_maps = []
    for i in range(N_CORES):
        r0 = i * RPC
        avail = min(IN_ROWS, H - r0)
        if avail == IN_ROWS:
            shard = X[r0 : r0 + IN_ROWS]
        else:
            shard = np.zeros((IN_ROWS, WIDTH), dtype=np.float32)
            shard[:avail] = X[r0 : r0 + avail]
        in_maps.append({"x": shard, "bands": bands, "bands2": bands2, "bias": bias})
    return in_maps


def _assemble(results):
    out = np.empty((OH, OW), dtype=np.float32)
    for i in range(N_CORES):
        r0 = i * RPC
        take = min(RPC, OH - r0)
        out[r0 : r0 + take] = results[i]["y"][:take]
    return out


def run(X, W, b, trace=False):
    nc = _get_nc()
    in_maps = _make_host_inputs(X, W, b)
    res = run_bass_kernel_spmd(nc, in_maps, list(range(N_CORES)), trace=trace)
    return _assemble(res.results), res


def kernel(X, W, b):
    out, _ = run(X, W, b)
    return out



# revision 2
# speedup vs baseline: 1.5605x; 1.5605x over previous
"""3x3 valid conv (single channel) on 8 TRN2 NeuronCores, fp16 I/O.

Strategy: memory-regime kernel, so halve HBM traffic by shipping X and Y as
fp16 (host converts; rel err ~6e-4, well under the 2e-2 gate). Per core the
conv is 3 banded matmuls per output tile accumulating in PSUM:
    out[m, c] = sum_dj (B_dj.T @ X_tile[:, c+dj])[m]
with B_dj[k, m] = W[k-m, dj] a [K, M] banded stationary built on host. fp16
matmuls run at 1 cyc/row directly on the DMA-loaded tiles (no f32r cast, so
the vector engine is free to share PSUM-drain duty with scalar).

Row split: 8 cores x 4 strips x 126 output rows = 4032 rows; the global
62-row tail strip (input rows 4032..4095) is column-split 8 ways and folded
2-up into 128 partitions (partitions 0..63 = cols [c0, c0+514), 64..127 =
cols [c0+512, c0+1026)) so one 512-wide matmul group covers 1024 output
columns of it. Tensor: ~100k cycles/core; DMA: ~16.6 MB/core.
"""

import sys

sys.path.insert(0, "/opt/trn_rl_repo")

import numpy as np
from concourse import bass, mybir
from concourse.bass_utils import run_bass_kernel_spmd
from concourse.tile import TileContext

F16 = mybir.dt.float16
F32 = mybir.dt.float32

H, WIDTH = 4096, 8192
KH, KW = 3, 3
OH, OW = H - KH + 1, WIDTH - KW + 1       # 4094, 8190
N_CORES = 8
N_STRIPS = 4                              # full strips per core
SRPC = N_STRIPS * 126                     # 504 strip-output rows per core
IN_ROWS = SRPC + KH - 1                   # 506 input rows per core
TAIL_R0 = N_CORES * SRPC                  # 4032: first tail output row
TAIL_ROWS = OH - TAIL_R0                  # 62 tail output rows
TAIL_COLS = 1024                          # tail output cols per core (folded 2x512)
N_GROUPS = 8                              # 1024-col PSUM groups per strip


def _split_multi_waits(nc, max_waits=1):
    # This container's walrus rejects >1 sync-wait command per instruction
    # (CoreV3 setupSyncWait). Tile attaches one wait per producing logical
    # processor to a single instruction; hoist the excess onto same-engine
    # Drain carriers inserted immediately before it.
    for fn in nc.m.functions:
        for bb in fn.blocks:
            out = []
            changed = False
            for inst in bb.instructions:
                si = inst.sync_info
                waits = list(si.on_wait) if si and si.on_wait else []
                if len(waits) > max_waits:
                    rest = waits[max_waits:]
                    for j in range(0, len(rest), max_waits):
                        carrier = mybir.InstDrain(
                            name=nc.get_next_instruction_name(), ins=[], outs=[]
                        )
                        carrier.engine = inst.engine
                        carrier.sync_info = mybir.SyncInfo(
                            on_wait=rest[j : j + max_waits], on_update=[]
                        )
                        out.append(carrier)
                    si.on_wait = waits[:max_waits]
                    changed = True
                out.append(inst)
            if changed:
                bb.instructions = out


def _build(split_waits=True):
    nc = bass.Bass()
    xm = nc.declare_dram_parameter("xm", [IN_ROWS, WIDTH], F16, isOutput=False)
    xt = nc.declare_dram_parameter("xt", [128, 514], F16, isOutput=False)
    bands = nc.declare_dram_parameter("bands", [128, 3 * 128], F16, isOutput=False)
    bandt = nc.declare_dram_parameter("bandt", [128, 3 * 128], F16, isOutput=False)
    bias = nc.declare_dram_parameter("bias", [128, 1], F32, isOutput=False)
    ym = nc.declare_dram_parameter("ym", [SRPC, OW], F16, isOutput=True)
    yt = nc.declare_dram_parameter("yt", [TAIL_ROWS, TAIL_COLS], F16, isOutput=True)

    ident = mybir.ActivationFunctionType.Identity

    with TileContext(nc) as tc:
        with (
            tc.tile_pool(name="const", bufs=1) as cpool,
            tc.tile_pool(name="xin", bufs=4) as xpool,
            tc.tile_pool(name="stage", bufs=3) as spool,
            tc.tile_pool(name="psum", bufs=4, space="PSUM") as ppool,
        ):
            # constants + tail input ride the (idle until stores) ACT HWDGE
            # ring so the strip loads below own the SP ring from t=0
            band_t = cpool.tile([128, 3 * 128], F16)
            nc.scalar.dma_start(out=band_t[:], in_=bands[:])
            bandt_t = cpool.tile([128, 3 * 128], F16)
            nc.scalar.dma_start(out=bandt_t[:], in_=bandt[:])
            bias_t = cpool.tile([128, 1], F32)
            nc.scalar.dma_start(out=bias_t[:], in_=bias[:])
            xt_t = cpool.tile([128, 514], F16)
            nc.scalar.dma_start(out=xt_t[:], in_=xt[:])

            strip_tiles = []
            for s in range(N_STRIPS):
                xr = xpool.tile([128, WIDTH], F16, tag="xt")
                # column halves so strip-s matmuls start after ~1 MB
                nc.sync.dma_start(
                    out=xr[:, 0:4096], in_=xm[126 * s : 126 * s + 128, 0:4096]
                )
                nc.sync.dma_start(
                    out=xr[:, 4096:WIDTH], in_=xm[126 * s : 126 * s + 128, 4096:WIDTH]
                )
                strip_tiles.append(xr)

            # ---- tail first: fills the ramp while strip 0 streams in ----
            ps = ppool.tile([128, 1024], F32, tag="ps")
            for dj in range(KW):
                nc.tensor.matmul(
                    ps[:126, 0:512],
                    bandt_t[:, dj * 128 : dj * 128 + 126],
                    xt_t[:, dj : dj + 512],
                    start=(dj == 0),
                    stop=(dj == KW - 1),
                )
            stage_t = spool.tile([128, 4096], F16, tag="stage")
            nc.scalar.activation(
                stage_t[:126, 0:512],
                ps[:126, 0:512],
                ident,
                bias=bias_t[:126, :],
                scale=1.0,
            )
            nc.scalar.dma_start(out=yt[:, 0:512], in_=stage_t[0:TAIL_ROWS, 0:512])
            nc.scalar.dma_start(out=yt[:, 512:1024], in_=stage_t[64 : 64 + TAIL_ROWS, 0:512])

            # ---- main strips ----
            for s in range(N_STRIPS):
                xr = strip_tiles[s]
                r0 = 126 * s
                for half in range(2):
                    stage = spool.tile([128, 4096], F16, tag="stage")
                    for gg in range(N_GROUPS // 2):
                        g = half * 4 + gg
                        ps = ppool.tile([128, 1024], F32, tag="ps")
                        for sub in range(2):
                            c0 = g * 1024 + sub * 512
                            n = min(512, OW - c0)
                            for dj in range(KW):
                                nc.tensor.matmul(
                                    ps[:126, sub * 512 : sub * 512 + n],
                                    band_t[:, dj * 128 : dj * 128 + 126],
                                    xr[:, c0 + dj : c0 + dj + n],
                                    start=(dj == 0),
                                    stop=(dj == KW - 1),
                                )
                        gw = min(1024, OW - g * 1024)
                        # alternate PSUM drains between scalar and vector so
                        # neither engine paces the pipeline
                        if g % 2 == 0:
                            nc.scalar.activation(
                                stage[:126, gg * 1024 : gg * 1024 + gw],
                                ps[:126, :gw],
                                ident,
                                bias=bias_t[:126, :],
                                scale=1.0,
                            )
                        else:
                            nc.vector.tensor_scalar_add(
                                stage[:126, gg * 1024 : gg * 1024 + gw],
                                ps[:126, :gw],
                                bias_t[:126, :],
                            )
                    hw_ = min(4096, OW - half * 4096)
                    nc.scalar.dma_start(
                        out=ym[r0 : r0 + 126, half * 4096 : half * 4096 + hw_],
                        in_=stage[0:126, :hw_],
                    )

    if split_waits:
        _split_multi_waits(nc)
    return nc


_NC_CACHE = None


def _get_nc():
    global _NC_CACHE
    if _NC_CACHE is None:
        _NC_CACHE = _build()
    return _NC_CACHE


def _make_host_inputs(X, W, b):
    Xh = np.asarray(X, dtype=np.float32).astype(np.float16)
    W = np.asarray(W, dtype=np.float32)
    b = np.asarray(b, dtype=np.float32)

    # main band: B[k, dj*128 + m] = W[k-m, dj] for 0 <= k-m < 3, m < 126
    bands = np.zeros((128, 3 * 128), dtype=np.float32)
    for dj in range(KW):
        for dk in range(KH):
            mm = np.arange(126)
            bands[mm + dk, dj * 128 + mm] = W[dk, dj]
    # tail band: same rule restricted to the two folded blocks
    # (k 0..63 -> m 0..61, k 64..127 -> m 64..125)
    bandt = np.zeros((128, 3 * 128), dtype=np.float32)
    for dj in range(KW):
        for dk in range(KH):
            mm = np.arange(TAIL_ROWS)
            bandt[mm + dk, dj * 128 + mm] = W[dk, dj]
            bandt[64 + mm + dk, dj * 128 + 64 + mm] = W[dk, dj]
    bands = bands.astype(np.float16)
    bandt = bandt.astype(np.float16)
    bias = np.full((128, 1), float(b[0]), dtype=np.float32)

    in_maps = []
    for i in range(N_CORES):
        r0 = i * SRPC
        shard = np.ascontiguousarray(Xh[r0 : r0 + IN_ROWS])
        # tail fold: partitions 0..63 = rows 4032..4095 cols [c0, c0+514),
        # partitions 64..127 = same rows cols [c0+512, c0+1026), zero-padded
        # past the right edge of X (core 7); the padded outputs aren't stored.
        c0 = i * TAIL_COLS
        take = min(514 + 512, WIDTH - c0)
        tpad = np.zeros((64, 514 + 512), dtype=np.float16)
        tpad[:, :take] = Xh[TAIL_R0 : TAIL_R0 + 64, c0 : c0 + take]
        xt = np.empty((128, 514), dtype=np.float16)
        xt[0:64] = tpad[:, 0:514]
        xt[64:128] = tpad[:, 512:1026]
        in_maps.append(
            {"xm": shard, "xt": xt, "bands": bands, "bandt": bandt, "bias": bias}
        )
    return in_maps


def _assemble(results):
    out = np.empty((OH, OW), dtype=np.float32)
    for i in range(N_CORES):
        r0 = i * SRPC
        out[r0 : r0 + SRPC] = results[i]["ym"].astype(np.float32)
        c0 = i * TAIL_COLS
        w = min(TAIL_COLS, OW - c0)
        out[TAIL_R0:OH, c0 : c0 + w] = results[i]["yt"][:, :w].astype(np.float32)
    return out


def run(X, W, b, trace=False):
    nc = _get_nc()
    in_maps = _make_host_inputs(X, W, b)
    res = run_bass_kernel_spmd(nc, in_maps, list(range(N_CORES)), trace=trace)
    return _assemble(res.results), res


def kernel(X, W, b):
    out, _ = run(X, W, b)
    return out


# revision 3
# speedup vs baseline: 1.7580x; 1.1266x over previous
"""3x3 valid conv (single channel) on 8 TRN2 NeuronCores, fp16 I/O.

Strategy: memory-regime kernel, so halve HBM traffic by shipping X and Y as
fp16 (host converts; rel err ~8e-4, well under the 2e-2 gate). Per core the
conv is 3 banded matmuls per output tile accumulating in PSUM:
    out[m, c] = sum_dj (B_dj.T @ X_tile[:, c+dj])[m]
with B_dj[k, m] = W[k-m, dj] a [K, M] banded stationary built on host. fp16
matmuls run at 1 cyc/row directly on the DMA-loaded tiles (no f32r cast, so
the vector engine is free to share PSUM-drain duty with scalar).

Row split: 8 cores x 4 strips x 126 output rows = 4032 rows; the global
62-row tail strip (input rows 4032..4095) is column-split 8 ways and folded
2-up into 128 partitions (partitions 0..63 = cols [c0, c0+514), 64..127 =
cols [c0+512, c0+1026)) so one 512-wide matmul group covers 1024 output
columns of it. Tensor: ~100k cycles/core; DMA: ~16.6 MB/core.

Latency hiding: consts ride the idle SWDGE (gpsimd) ring so the SP ring
streams X from t=0 in 513 KB chunks; PE runs throwaway matmuls on a memset
scratch tile during the first chunk's flight so the HAM clock-gate is warm
(2.4 GHz) when real work arrives; a dummy activation preloads the ACT
table; the tail strip sits mid-stream so its store isn't on the critical
path. PSUM drains alternate scalar/vector so neither engine paces.
"""

import sys

sys.path.insert(0, "/opt/trn_rl_repo")

import numpy as np
from concourse import bass, mybir
from concourse.bass_utils import run_bass_kernel_spmd
from concourse.tile import TileContext

F16 = mybir.dt.float16
F32 = mybir.dt.float32

H, WIDTH = 4096, 8192
KH, KW = 3, 3
OH, OW = H - KH + 1, WIDTH - KW + 1       # 4094, 8190
N_CORES = 8
N_STRIPS = 4                              # full strips per core
SRPC = N_STRIPS * 126                     # 504 strip-output rows per core
IN_ROWS = SRPC + KH - 1                   # 506 input rows per core
TAIL_R0 = N_CORES * SRPC                  # 4032: first tail output row
TAIL_ROWS = OH - TAIL_R0                  # 62 tail output rows
TAIL_COLS = 1024                          # tail output cols per core (folded 2x512)
N_GROUPS = 8                              # 1024-col PSUM groups per strip
N_CHUNKS = 4                              # load chunks per strip
CHUNK = 2050                              # chunk width (last one 2046)
N_WARM = 30                               # HAM warmup matmuls (N=128 on scratch)


def _split_multi_waits(nc, max_waits=1):
    # This container's walrus rejects >1 sync-wait command per instruction
    # (CoreV3 setupSyncWait). Tile attaches one wait per producing logical
    # processor to a single instruction; hoist the excess onto same-engine
    # Drain carriers inserted immediately before it.
    for fn in nc.m.functions:
        for bb in fn.blocks:
            out = []
            changed = False
            for inst in bb.instructions:
                si = inst.sync_info
                waits = list(si.on_wait) if si and si.on_wait else []
                if len(waits) > max_waits:
                    rest = waits[max_waits:]
                    for j in range(0, len(rest), max_waits):
                        carrier = mybir.InstDrain(
                            name=nc.get_next_instruction_name(), ins=[], outs=[]
                        )
                        carrier.engine = inst.engine
                        carrier.sync_info = mybir.SyncInfo(
                            on_wait=rest[j : j + max_waits], on_update=[]
                        )
                        out.append(carrier)
                    si.on_wait = waits[:max_waits]
                    changed = True
                out.append(inst)
            if changed:
                bb.instructions = out


def _build(split_waits=True):
    nc = bass.Bass()
    xm = nc.declare_dram_parameter("xm", [IN_ROWS, WIDTH], F16, isOutput=False)
    xt = nc.declare_dram_parameter("xt", [128, 514], F16, isOutput=False)
    bands = nc.declare_dram_parameter("bands", [128, 3 * 128], F16, isOutput=False)
    bandt = nc.declare_dram_parameter("bandt", [128, 3 * 128], F16, isOutput=False)
    bias = nc.declare_dram_parameter("bias", [128, 1], F32, isOutput=False)
    ym = nc.declare_dram_parameter("ym", [SRPC, OW], F16, isOutput=True)
    yt = nc.declare_dram_parameter("yt", [TAIL_ROWS, TAIL_COLS], F16, isOutput=True)

    ident = mybir.ActivationFunctionType.Identity

    with TileContext(nc) as tc:
        with (
            tc.tile_pool(name="const", bufs=1) as cpool,
            tc.tile_pool(name="xin", bufs=4) as xpool,
            tc.tile_pool(name="stage", bufs=3) as spool,
            tc.tile_pool(name="psum", bufs=4, space="PSUM") as ppool,
        ):
            # consts ride the otherwise-idle SWDGE ring: they land in ~2-3us
            # while the SP ring streams X chunks uncontended
            band_t = cpool.tile([128, 3 * 128], F16)
            nc.gpsimd.dma_start(out=band_t[:], in_=bands[:])
            bandt_t = cpool.tile([128, 3 * 128], F16)
            nc.gpsimd.dma_start(out=bandt_t[:], in_=bandt[:])
            bias_t = cpool.tile([128, 1], F32)
            nc.gpsimd.dma_start(out=bias_t[:], in_=bias[:])
            xt_t = cpool.tile([128, 514], F16)
            nc.gpsimd.dma_start(out=xt_t[:], in_=xt[:])

            scratch = cpool.tile([128, 128], F16)
            nc.vector.memset(scratch[:], 0.0)
            scratch2 = cpool.tile([128, 16], F32)

            strip_tiles = []
            for s in range(N_STRIPS):
                xr = xpool.tile([128, WIDTH], F16, tag="xt")
                for k in range(N_CHUNKS):
                    c0 = k * CHUNK
                    w = min(CHUNK, WIDTH - c0)
                    nc.sync.dma_start(
                        out=xr[:, c0 : c0 + w],
                        in_=xm[126 * s : 126 * s + 128, c0 : c0 + w],
                    )
                strip_tiles.append(xr)

            # HAM warmup: throwaway N=128 matmuls on the scratch tile keep
            # the PE busy while chunk 0 is in flight, so the clock gate is
            # at 8/8 (2.4 GHz) when the real stream begins. The ACT table
            # preload rides the same scratch.
            warm_ps = ppool.tile([128, 1024], F32, tag="ps")
            for i in range(N_WARM):
                nc.tensor.matmul(
                    warm_ps[:126, 0:128],
                    scratch[:, 0:126],
                    scratch[:, 0:128],
                    start=True,
                    stop=True,
                )
            nc.scalar.activation(
                scratch2[:, 0:16], scratch[:, 0:16], ident, bias=0.0, scale=1.0
            )

            def do_tail():
                ps = ppool.tile([128, 1024], F32, tag="ps")
                for dj in range(KW):
                    nc.tensor.matmul(
                        ps[:126, 0:512],
                        bandt_t[:, dj * 128 : dj * 128 + 126],
                        xt_t[:, dj : dj + 512],
                        start=(dj == 0),
                        stop=(dj == KW - 1),
                    )
                stage_t = spool.tile([128, 4096], F16, tag="stage")
                nc.scalar.activation(
                    stage_t[:126, 0:512],
                    ps[:126, 0:512],
                    ident,
                    bias=bias_t[:126, :],
                    scale=1.0,
                )
                nc.scalar.dma_start(out=yt[:, 0:512], in_=stage_t[0:TAIL_ROWS, 0:512])
                nc.scalar.dma_start(
                    out=yt[:, 512:1024], in_=stage_t[64 : 64 + TAIL_ROWS, 0:512]
                )

            for s in range(N_STRIPS):
                xr = strip_tiles[s]
                r0 = 126 * s
                for half in range(2):
                    stage = spool.tile([128, 4096], F16, tag="stage")
                    for gg in range(N_GROUPS // 2):
                        g = half * 4 + gg
                        ps = ppool.tile([128, 1024], F32, tag="ps")
                        for sub in range(2):
                            c0 = g * 1024 + sub * 512
                            n = min(512, OW - c0)
                            for dj in range(KW):
                                nc.tensor.matmul(
                                    ps[:126, sub * 512 : sub * 512 + n],
                                    band_t[:, dj * 128 : dj * 128 + 126],
                                    xr[:, c0 + dj : c0 + dj + n],
                                    start=(dj == 0),
                                    stop=(dj == KW - 1),
                                )
                        gw = min(1024, OW - g * 1024)
                        # alternate PSUM drains between scalar and vector so
                        # neither engine paces the pipeline
                        if g % 2 == 0:
                            nc.scalar.activation(
                                stage[:126, gg * 1024 : gg * 1024 + gw],
                                ps[:126, :gw],
                                ident,
                                bias=bias_t[:126, :],
                                scale=1.0,
                            )
                        else:
                            nc.vector.tensor_scalar_add(
                                stage[:126, gg * 1024 : gg * 1024 + gw],
                                ps[:126, :gw],
                                bias_t[:126, :],
                            )
                    hw_ = min(4096, OW - half * 4096)
                    nc.scalar.dma_start(
                        out=ym[r0 : r0 + 126, half * 4096 : half * 4096 + hw_],
                        in_=stage[0:126, :hw_],
                    )
                if s == 1:
                    # tail sits mid-stream: its inputs landed early on the
                    # SWDGE ring and its store stays off the critical path
                    do_tail()

    if split_waits:
        _split_multi_waits(nc)
    return nc


_NC_CACHE = None


def _get_nc():
    global _NC_CACHE
    if _NC_CACHE is None:
        _NC_CACHE = _build()
    return _NC_CACHE


def _make_host_inputs(X, W, b):
    Xh = np.asarray(X, dtype=np.float32).astype(np.float16)
    W = np.asarray(W, dtype=np.float32)
    b = np.asarray(b, dtype=np.float32)

    # main band: B[k, dj*128 + m] = W[k-m, dj] for 0 <= k-m < 3, m < 126
    bands = np.zeros((128, 3 * 128), dtype=np.float32)
    for dj in range(KW):
        for dk in range(KH):
            mm = np.arange(126)
            bands[mm + dk, dj * 128 + mm] = W[dk, dj]
    # tail band: same rule restricted to the two folded blocks
    # (k 0..63 -> m 0..61, k 64..127 -> m 64..125)
    bandt = np.zeros((128, 3 * 128), dtype=np.float32)
    for dj in range(KW):
        for dk in range(KH):
            mm = np.arange(TAIL_ROWS)
            bandt[mm + dk, dj * 128 + mm] = W[dk, dj]
            bandt[64 + mm + dk, dj * 128 + 64 + mm] = W[dk, dj]
    bands = bands.astype(np.float16)
    bandt = bandt.astype(np.float16)
    bias = np.full((128, 1), float(b[0]), dtype=np.float32)

    in_maps = []
    for i in range(N_CORES):
        r0 = i * SRPC
        shard = np.ascontiguousarray(Xh[r0 : r0 + IN_ROWS])
        # tail fold: partitions 0..63 = rows 4032..4095 cols [c0, c0+514),
        # partitions 64..127 = same rows cols [c0+512, c0+1026), zero-padded
        # past the right edge of X (core 7); the padded outputs aren't stored.
        c0 = i * TAIL_COLS
        take = min(514 + 512, WIDTH - c0)
        tpad = np.zeros((64, 514 + 512), dtype=np.float16)
        tpad[:, :take] = Xh[TAIL_R0 : TAIL_R0 + 64, c0 : c0 + take]
        xt = np.empty((128, 514), dtype=np.float16)
        xt[0:64] = tpad[:, 0:514]
        xt[64:128] = tpad[:, 512:1026]
        in_maps.append(
            {"xm": shard, "xt": xt, "bands": bands, "bandt": bandt, "bias": bias}
        )
    return in_maps


def _assemble(results):
    out = np.empty((OH, OW), dtype=np.float32)
    for i in range(N_CORES):
        r0 = i * SRPC
        out[r0 : r0 + SRPC] = results[i]["ym"].astype(np.float32)
        c0 = i * TAIL_COLS
        w = min(TAIL_COLS, OW - c0)
        out[TAIL_R0:OH, c0 : c0 + w] = results[i]["yt"][:, :w].astype(np.float32)
    return out


def run(X, W, b, trace=False):
    nc = _get_nc()
    in_maps = _make_host_inputs(X, W, b)
    res = run_bass_kernel_spmd(nc, in_maps, list(range(N_CORES)), trace=trace)
    return _assemble(res.results), res


def kernel(X, W, b):
    out, _ = run(X, W, b)
    return out


# revision 6
# speedup vs baseline: 1.8891x; 1.0746x over previous
"""3x3 valid conv (single channel) on 8 TRN2 NeuronCores, fp16 I/O.

Strategy: memory-regime kernel, so halve HBM traffic by shipping X and Y as
fp16 (host converts; rel err ~8e-4, well under the 2e-2 gate). Per core the
conv is 3 banded matmuls per output tile accumulating in PSUM:
    out[m, c] = sum_dj (B_dj.T @ X_tile[:, c+dj])[m]
with B_dj[k, m] = W[k-m, dj] a [K, M] banded stationary built on host. fp16
matmuls run at 1 cyc/row directly on the DMA-loaded tiles (no f32r cast, so
the vector engine is free to share PSUM-drain duty with scalar).

Row split: 8 cores x 4 strips x 126 output rows = 4032 rows; the global
62-row tail strip (input rows 4032..4095) is column-split 8 ways and folded
2-up into 128 partitions (partitions 0..63 = cols [c0, c0+514), 64..127 =
cols [c0+512, c0+1026)) so one 512-wide matmul group covers 1024 output
columns of it. Tensor: ~100k cycles/core; DMA: ~16.6 MB/core.

Latency hiding: consts ride the idle SWDGE (gpsimd) ring so the SP ring
streams X from t=0 in 513 KB chunks; PE runs throwaway matmuls on a memset
scratch tile during the first chunk's flight so the HAM clock-gate is warm
(2.4 GHz) when real work arrives; a dummy activation preloads the ACT
table; the tail strip sits mid-stream so its store isn't on the critical
path. PSUM drains alternate scalar/vector so neither engine paces.
"""

import sys

sys.path.insert(0, "/opt/trn_rl_repo")

import numpy as np
from concourse import bass, mybir
from concourse.bass_utils import run_bass_kernel_spmd
from concourse.tile import TileContext

F16 = mybir.dt.float16
F32 = mybir.dt.float32

H, WIDTH = 4096, 8192
KH, KW = 3, 3
OH, OW = H - KH + 1, WIDTH - KW + 1       # 4094, 8190
N_CORES = 8
N_STRIPS = 4                              # full strips per core
SRPC = N_STRIPS * 126                     # 504 strip-output rows per core
IN_ROWS = SRPC + KH - 1                   # 506 input rows per core
TAIL_R0 = N_CORES * SRPC                  # 4032: first tail output row
TAIL_ROWS = OH - TAIL_R0                  # 62 tail output rows
TAIL_COLS = 1024                          # tail output cols per core (folded 2x512)
N_GROUPS = 8                              # 1024-col PSUM groups per strip
# strip-0 load chunk boundaries: small first chunk so the real matmul stream
# starts ASAP; later strips load in halves (fewer DMAs -> less DMAHW-lane
# serialization against the store DMAs, which share the 8 lanes)
S0_CHUNKS = [0, 1026, 3076, 5126, 7176, WIDTH]
N_WARM = 34                               # HAM warmup matmuls (N=128 on scratch)


def _split_multi_waits(nc, max_waits=1):
    # This container's walrus rejects >1 sync-wait command per instruction
    # (CoreV3 setupSyncWait). Tile attaches one wait per producing logical
    # processor to a single instruction; hoist the excess onto same-engine
    # Drain carriers inserted immediately before it.
    for fn in nc.m.functions:
        for bb in fn.blocks:
            out = []
            changed = False
            for inst in bb.instructions:
                si = inst.sync_info
                waits = list(si.on_wait) if si and si.on_wait else []
                if len(waits) > max_waits:
                    rest = waits[max_waits:]
                    for j in range(0, len(rest), max_waits):
                        carrier = mybir.InstDrain(
                            name=nc.get_next_instruction_name(), ins=[], outs=[]
                        )
                        carrier.engine = inst.engine
                        carrier.sync_info = mybir.SyncInfo(
                            on_wait=rest[j : j + max_waits], on_update=[]
                        )
                        out.append(carrier)
                    si.on_wait = waits[:max_waits]
                    changed = True
                out.append(inst)
            if changed:
                bb.instructions = out


def _build(split_waits=True):
    nc = bass.Bass()
    xm = nc.declare_dram_parameter("xm", [IN_ROWS, WIDTH], F16, isOutput=False)
    xt = nc.declare_dram_parameter("xt", [128, 514], F16, isOutput=False)
    bands = nc.declare_dram_parameter("bands", [128, 3 * 128], F16, isOutput=False)
    bandt = nc.declare_dram_parameter("bandt", [128, 3 * 128], F16, isOutput=False)
    bias = nc.declare_dram_parameter("bias", [128, 1], F32, isOutput=False)
    ym = nc.declare_dram_parameter("ym", [SRPC, OW], F16, isOutput=True)
    yt = nc.declare_dram_parameter("yt", [TAIL_ROWS, TAIL_COLS], F16, isOutput=True)

    ident = mybir.ActivationFunctionType.Identity

    with TileContext(nc) as tc:
        with (
            tc.tile_pool(name="const", bufs=1) as cpool,
            tc.tile_pool(name="xin", bufs=4) as xpool,
            tc.tile_pool(name="stage", bufs=3) as spool,
            tc.tile_pool(name="psum", bufs=4, space="PSUM") as ppool,
        ):
            # consts ride the otherwise-idle SWDGE ring: they land in ~2-3us
            # while the SP ring streams X chunks uncontended
            band_t = cpool.tile([128, 3 * 128], F16)
            nc.gpsimd.dma_start(out=band_t[:], in_=bands[:])
            bandt_t = cpool.tile([128, 3 * 128], F16)
            nc.gpsimd.dma_start(out=bandt_t[:], in_=bandt[:])
            bias_t = cpool.tile([128, 1], F32)
            nc.gpsimd.dma_start(out=bias_t[:], in_=bias[:])
            xt_t = cpool.tile([128, 514], F16)
            nc.gpsimd.dma_start(out=xt_t[:], in_=xt[:])

            scratch = cpool.tile([128, 128], F16)
            nc.vector.memset(scratch[:], 0.0)
            scratch2 = cpool.tile([128, 16], F32)

            strip_tiles = []
            for s in range(N_STRIPS):
                xr = xpool.tile([128, WIDTH], F16, tag="xt")
                bounds = S0_CHUNKS if s == 0 else [0, 4096, WIDTH]
                for c0, c1 in zip(bounds, bounds[1:]):
                    nc.sync.dma_start(
                        out=xr[:, c0:c1],
                        in_=xm[126 * s : 126 * s + 128, c0:c1],
                    )
                strip_tiles.append(xr)

            # HAM warmup: throwaway N=128 matmuls on the scratch tile keep
            # the PE busy while chunk 0 is in flight, so the clock gate is
            # at 8/8 (2.4 GHz) when the real stream begins. The ACT table
            # preload rides the same scratch.
            warm_ps = ppool.tile([128, 1024], F32, tag="ps")
            for i in range(N_WARM):
                nc.tensor.matmul(
                    warm_ps[:126, 0:128],
                    scratch[:, 0:126],
                    scratch[:, 0:128],
                    start=True,
                    stop=True,
                )
            nc.scalar.activation(
                scratch2[:, 0:16], scratch[:, 0:16], ident, bias=0.0, scale=1.0
            )

            def do_tail():
                ps = ppool.tile([128, 1024], F32, tag="ps")
                for dj in range(KW):
                    nc.tensor.matmul(
                        ps[:126, 0:512],
                        bandt_t[:, dj * 128 : dj * 128 + 126],
                        xt_t[:, dj : dj + 512],
                        start=(dj == 0),
                        stop=(dj == KW - 1),
                    )
                stage_t = spool.tile([128, 4096], F16, tag="stage")
                nc.scalar.activation(
                    stage_t[:126, 0:512],
                    ps[:126, 0:512],
                    ident,
                    bias=bias_t[:126, :],
                    scale=1.0,
                )
                nc.scalar.dma_start(out=yt[:, 0:512], in_=stage_t[0:TAIL_ROWS, 0:512])
                nc.scalar.dma_start(
                    out=yt[:, 512:1024], in_=stage_t[64 : 64 + TAIL_ROWS, 0:512]
                )

            for s in range(N_STRIPS):
                xr = strip_tiles[s]
                r0 = 126 * s
                for half in range(2):
                    stage = spool.tile([128, 4096], F16, tag="stage")
                    for gg in range(N_GROUPS // 2):
                        g = half * 4 + gg
                        ps = ppool.tile([128, 1024], F32, tag="ps")
                        for sub in range(2):
                            c0 = g * 1024 + sub * 512
                            n = min(512, OW - c0)
                            for dj in range(KW):
                                nc.tensor.matmul(
                                    ps[:126, sub * 512 : sub * 512 + n],
                                    band_t[:, dj * 128 : dj * 128 + 126],
                                    xr[:, c0 + dj : c0 + dj + n],
                                    start=(dj == 0),
                                    stop=(dj == KW - 1),
                                )
                        gw = min(1024, OW - g * 1024)
                        # alternate PSUM drains between scalar and vector so
                        # neither engine paces the pipeline
                        if g % 2 == 0:
                            nc.scalar.activation(
                                stage[:126, gg * 1024 : gg * 1024 + gw],
                                ps[:126, :gw],
                                ident,
                                bias=bias_t[:126, :],
                                scale=1.0,
                            )
                        else:
                            nc.vector.tensor_scalar_add(
                                stage[:126, gg * 1024 : gg * 1024 + gw],
                                ps[:126, :gw],
                                bias_t[:126, :],
                            )
                        if s == N_STRIPS - 1 and gg % 2 == 1:
                            # last strip: store per 2 drains so the final
                            # store is small and off the critical path
                            q0 = half * 4096 + (gg - 1) * 1024
                            qw = min(2048, OW - q0)
                            nc.scalar.dma_start(
                                out=ym[r0 : r0 + 126, q0 : q0 + qw],
                                in_=stage[0:126, (gg - 1) * 1024 : (gg - 1) * 1024 + qw],
                            )
                    if s < N_STRIPS - 1:
                        hw_ = min(4096, OW - half * 4096)
                        nc.scalar.dma_start(
                            out=ym[r0 : r0 + 126, half * 4096 : half * 4096 + hw_],
                            in_=stage[0:126, :hw_],
                        )
                if s == 1:
                    # tail sits mid-stream: its inputs landed early on the
                    # SWDGE ring and its store stays off the critical path
                    do_tail()

    if split_waits:
        _split_multi_waits(nc)
    return nc


_NC_CACHE = None


def _get_nc():
    global _NC_CACHE
    if _NC_CACHE is None:
        _NC_CACHE = _build()
    return _NC_CACHE


def _make_host_inputs(X, W, b):
    Xh = np.asarray(X, dtype=np.float32).astype(np.float16)
    W = np.asarray(W, dtype=np.float32)
    b = np.asarray(b, dtype=np.float32)

    # main band: B[k, dj*128 + m] = W[k-m, dj] for 0 <= k-m < 3, m < 126
    bands = np.zeros((128, 3 * 128), dtype=np.float32)
    for dj in range(KW):
        for dk in range(KH):
            mm = np.arange(126)
            bands[mm + dk, dj * 128 + mm] = W[dk, dj]
    # tail band: same rule restricted to the two folded blocks
    # (k 0..63 -> m 0..61, k 64..127 -> m 64..125)
    bandt = np.zeros((128, 3 * 128), dtype=np.float32)
    for dj in range(KW):
        for dk in range(KH):
            mm = np.arange(TAIL_ROWS)
            bandt[mm + dk, dj * 128 + mm] = W[dk, dj]
            bandt[64 + mm + dk, dj * 128 + 64 + mm] = W[dk, dj]
    bands = bands.astype(np.float16)
    bandt = bandt.astype(np.float16)
    bias = np.full((128, 1), float(b[0]), dtype=np.float32)

    in_maps = []
    for i in range(N_CORES):
        r0 = i * SRPC
        shard = np.ascontiguousarray(Xh[r0 : r0 + IN_ROWS])
        # tail fold: partitions 0..63 = rows 4032..4095 cols [c0, c0+514),
        # partitions 64..127 = same rows cols [c0+512, c0+1026), zero-padded
        # past the right edge of X (core 7); the padded outputs aren't stored.
        c0 = i * TAIL_COLS
        take = min(514 + 512, WIDTH - c0)
        tpad = np.zeros((64, 514 + 512), dtype=np.float16)
        tpad[:, :take] = Xh[TAIL_R0 : TAIL_R0 + 64, c0 : c0 + take]
        xt = np.empty((128, 514), dtype=np.float16)
        xt[0:64] = tpad[:, 0:514]
        xt[64:128] = tpad[:, 512:1026]
        in_maps.append(
            {"xm": shard, "xt": xt, "bands": bands, "bandt": bandt, "bias": bias}
        )
    return in_maps


def _assemble(results):
    out = np.empty((OH, OW), dtype=np.float32)
    for i in range(N_CORES):
        r0 = i * SRPC
        out[r0 : r0 + SRPC] = results[i]["ym"].astype(np.float32)
        c0 = i * TAIL_COLS
        w = min(TAIL_COLS, OW - c0)
        out[TAIL_R0:OH, c0 : c0 + w] = results[i]["yt"][:, :w].astype(np.float32)
    return out


def run(X, W, b, trace=False):
    nc = _get_nc()
    in_maps = _make_host_inputs(X, W, b)
    res = run_bass_kernel_spmd(nc, in_maps, list(range(N_CORES)), trace=trace)
    return _assemble(res.results), res


def kernel(X, W, b):
    out, _ = run(X, W, b)
    return out
